# revision 1
# baseline (speedup 1.0000x reference)
"""DegreeAwareEdgeEncoder Trainium2 kernel (8 NeuronCores, Bass/Tile).

Sharding strategy (host side, inside kernel()):
  Edges are distributed core- and partition-parallel by *source-node range*
  (vertex-range / CSR-style partitioning): virtual node space of
  102400 = 8 cores x 128 partitions x 100 nodes; the edges whose src falls in
  partition slab (c, p)'s 100-node range are delivered to that slab, sorted by
  src.  A second copy of the dst column is distributed the same way by
  *dst*-range.  All arithmetic happens on the device:
    - out-degree per edge: per-partition local histogram of the slab's src
      values over its 100-node range (DVE dense compare; exact because all
      edges of one src node land in one slab) followed by an in-slab lookup.
    - in-degree: same histogram machinery on the dst-bucketed copy, AllGather
      of the 8 per-core [12800] slices into the full [102400] degree vector,
      int8 quad table, then a per-edge GPSIMD ap_gather + quad select.
    - output rows: du*A' + dv*B' + b with A'=W0+W2, B'=W1+W2 (PE computes the
      3xEMB coefficient rows; DVE does the broadcast expansion), written back
      as [E, 32] f32.
  The host only buckets/sorts (data layout), pads with sentinel edges, and
  inverts the layout permutation on the returned rows.
"""

import numpy as np

import concourse.bass as bass
import concourse.mybir as mybir
import concourse.tile as tile
from concourse.tile_rust import add_dep_helper
from concourse import bacc
from concourse.library_config import ap_gather as APG_LIB
from concourse.bass_utils import run_bass_kernel_spmd

# ---- constants ----
N_NODES = 100_000
N_EDGES = 3_200_000
EMB = 32
NCORES = 8
P = 128
BPP = 100                  # nodes per partition slab
NV = NCORES * P * BPP      # 102400 virtual nodes
RC = P * BPP               # 12800 nodes per core
T = 3584                   # slab capacity (cols per partition)
TQ = NV // 4               # 25600 int8 quads in the gather table
GCH = 16                   # ap_gather chunks
TCH = T // GCH             # 224 idx cols per chunk
NIC = TCH * 16             # 3584 idxs per chunk per q7 core
XCH = 56                   # expansion chunk cols
BCH = 4                    # hist bins per chunk
PAD_SENTINEL = BPP         # local value that never matches bins 0..99

f32 = mybir.dt.float32
i32 = mybir.dt.int32
i16 = mybir.dt.int16
i8 = mybir.dt.int8
AO = mybir.AluOpType

_CACHE = {}


def _build():
    nc = bacc.Bacc("TRN2", target_bir_lowering=False, debug=False,
                   num_devices=NCORES)

    psrc = nc.dram_tensor("psrc", [P, T], i32, kind="ExternalInput")
    pdst = nc.dram_tensor("pdst", [P, T], i32, kind="ExternalInput")
    sdst = nc.dram_tensor("sdst", [P, T], i32, kind="ExternalInput")
    wb_in = nc.dram_tensor("wb", [4, EMB], f32, kind="ExternalInput")
    mmat = nc.dram_tensor("mmat", [4, 4], f32, kind="ExternalInput")
    basec = nc.dram_tensor("basec", [P, 1], f32, kind="ExternalInput")
    iotab = nc.dram_tensor("iotab", [P, BPP], f32, kind="ExternalInput")
    smask = nc.dram_tensor("smask", [P, 16], f32, kind="ExternalInput")
    out = nc.dram_tensor("out", [P, T, EMB], f32, kind="ExternalOutput")

    slice_d = nc.dram_tensor("slice_d", [RC], f32)
    full_d = nc.dram_tensor("full_d", [NV], f32, addr_space="Shared")
    deg8_d = nc.dram_tensor("deg8_d", [NV], i8)
    abb_d = nc.dram_tensor("abb_d", [4, EMB], f32)

    with tile.TileContext(nc) as tc, nc.allow_low_precision(
            reason="all values are small integers, exact in bf16"):
        with (
            tc.tile_pool(name="main", bufs=1) as pool,
            tc.tile_pool(name="psum", bufs=1, space="PSUM") as psum,
        ):
            # ---- small constant inputs ----
            wb_t = pool.tile([4, EMB], f32)
            mm_t = pool.tile([4, 4], f32)
            basec_t = pool.tile([P, 1], f32)
            iotab_t = pool.tile([P, BPP], f32)
            nc.sync.dma_start(out=wb_t[:], in_=wb_in[:])
            nc.sync.dma_start(out=mm_t[:], in_=mmat[:])
            nc.sync.dma_start(out=basec_t[:], in_=basec[:])
            nc.sync.dma_start(out=iotab_t[:], in_=iotab[:])
            smask_t = pool.tile([P, 16], f32)
            nc.sync.dma_start(out=smask_t[:], in_=smask[:])

            # ---- coefficient rows: [A'; B'; b; 0] = mmat^T @ [W; b] ----
            abb_ps = psum.tile([4, EMB], f32)
            nc.tensor.matmul(out=abb_ps[:], lhsT=mm_t[:], rhs=wb_t[:],
                             start=True, stop=True)
            abb_t = pool.tile([4, EMB], f32)
            nc.vector.tensor_copy(out=abb_t[:], in_=abb_ps[:])
            nc.sync.dma_start(out=abb_d[:], in_=abb_t[:])
            arep = pool.tile([P, EMB], f32)
            brep = pool.tile([P, EMB], f32)
            crep = pool.tile([P, EMB], f32)
            nc.sync.dma_start(out=arep[:], in_=abb_d[0:1, :].to_broadcast([P, EMB]))
            nc.sync.dma_start(out=brep[:], in_=abb_d[1:2, :].to_broadcast([P, EMB]))
            nc.sync.dma_start(out=crep[:], in_=abb_d[2:3, :].to_broadcast([P, EMB]))

            hist_dst = pool.tile([P, BPP], f32)
            hist_src = pool.tile([P, BPP], f32)

            def dense_hist(vn, hist):
                for bc in range(BPP // BCH):
                    cmp = pool.tile([P, BCH, T], f32, tag="slotT")
                    nc.vector.tensor_tensor(
                        out=cmp[:],
                        in0=vn[:][:, None, :].to_broadcast([P, BCH, T]),
                        in1=iotab_t[:, BCH * bc:BCH * (bc + 1)][:, :, None]
                            .to_broadcast([P, BCH, T]),
                        op=AO.is_equal)
                    nc.vector.tensor_reduce(
                        out=hist[:, BCH * bc:BCH * (bc + 1)],
                        in_=cmp[:], op=AO.add, axis=mybir.AxisListType.X)

            # ---- dst histogram (slot B holds vndst) ----
            sdst_t = pool.tile([P, T], i32, tag="slotA")
            nc.sync.dma_start(out=sdst_t[:], in_=sdst[:])
            vndst = pool.tile([P, T], f32, tag="slotB")
            nc.vector.tensor_copy(out=vndst[:], in_=sdst_t[:])
            nc.vector.scalar_tensor_tensor(
                out=vndst[:], in0=vndst[:], scalar=basec_t[:, 0:1],
                in1=vndst[:], op0=AO.subtract, op1=AO.bypass)
            dense_hist(vndst, hist_dst)

            # ---- allgather in-degree slices ----
            nc.sync.dma_start(out=slice_d[:].rearrange("(p c) -> p c", p=P),
                              in_=hist_dst[:])
            nc.gpsimd.collective_compute(
                "AllGather", AO.bypass,
                replica_groups=[list(range(NCORES))],
                ins=[slice_d[:]], outs=[full_d[:]])

            # ---- src histogram + du lookup (slot B holds vnsrc) ----
            psrc_t = pool.tile([P, T], i32, tag="slotA")
            nc.sync.dma_start(out=psrc_t[:], in_=psrc[:])
            vnsrc = pool.tile([P, T], f32, tag="slotB")
            nc.vector.tensor_copy(out=vnsrc[:], in_=psrc_t[:])
            nc.vector.scalar_tensor_tensor(
                out=vnsrc[:], in0=vnsrc[:], scalar=basec_t[:, 0:1],
                in1=vnsrc[:], op0=AO.subtract, op1=AO.bypass)
            dense_hist(vnsrc, hist_src)
            du_t = pool.tile([P, T], mybir.dt.bfloat16)
            nc.vector.memset(du_t[:], 0.0)
            for bc in range(BPP // BCH):
                cmp = pool.tile([P, BCH, T], f32, tag="slotT")
                nc.vector.tensor_tensor(
                    out=cmp[:],
                    in0=vnsrc[:][:, None, :].to_broadcast([P, BCH, T]),
                    in1=iotab_t[:, BCH * bc:BCH * (bc + 1)][:, :, None]
                        .to_broadcast([P, BCH, T]),
                    op=AO.is_equal)
                for j in range(BCH):
                    b = BCH * bc + j
                    nc.vector.scalar_tensor_tensor(
                        out=du_t[:], in0=cmp[:, j, :],
                        scalar=hist_src[:, b:b + 1], in1=du_t[:],
                        op0=AO.mult, op1=AO.add)

            # ---- int8 degree table, replicated per partition ----
            degf = pool.tile([P, NV // P], f32, tag="slotE")
            nc.sync.dma_start(out=degf[:],
                              in_=full_d[:].rearrange("(p c) -> p c", p=P))
            deg8s = pool.tile([P, NV // P], i8, tag="wsel")
            nc.vector.tensor_copy(out=deg8s[:], in_=degf[:])
            nc.sync.dma_start(out=deg8_d[:].rearrange("(p c) -> p c", p=P),
                              in_=deg8s[:])
            table8 = pool.tile([P, NV], i8, tag="slotT")
            nc.sync.dma_start(
                out=table8[:],
                in_=deg8_d[:][None, :].to_broadcast([P, NV]))

            # ---- gather indices: quad idx int16 + remainder ----
            pdst_t = pool.tile([P, T], i32, tag="slotA")
            nc.sync.dma_start(out=pdst_t[:], in_=pdst[:])
            pf = pool.tile([P, T], f32, tag="slotB")
            nc.vector.tensor_copy(out=pf[:], in_=pdst_t[:])
            qf = pool.tile([P, T], f32, tag="slotE")
            nc.vector.tensor_scalar(out=qf[:], in0=pf[:], scalar1=0.25,
                                    scalar2=-0.375, op0=AO.mult, op1=AO.add)
            idxw = pool.tile([P, T], i16)
            nc.vector.tensor_copy(out=idxw[:], in_=qf[:])   # round -> exact quad
            qround = pool.tile([P, T], f32, tag="slotE")
            nc.vector.tensor_copy(out=qround[:], in_=idxw[:])
            rem = pf                                        # dst - 4*quad in 0..3
            nc.vector.scalar_tensor_tensor(
                out=rem[:], in0=qround[:], scalar=-4.0, in1=pf[:],
                op0=AO.mult, op1=AO.add)

            # ---- per-edge in-degree gather (GPSIMD ap_gather, int8 quads) ----
            lib_inst = nc.gpsimd.load_library(APG_LIB)
            tbl_q = table8[:].rearrange("p (q d) -> p q d", d=4)
            dv_t = pool.tile([P, T], mybir.dt.bfloat16)
            iota4 = pool.tile([P, 4], f32)
            for r in range(4):
                nc.vector.memset(iota4[:, r:r + 1], float(r))
            bf = mybir.dt.bfloat16
            for g in range(GCH):
                gsl = slice(g * TCH, (g + 1) * TCH)
                qgat = pool.tile([P, NIC, 4], i8, tag="slotA")
                gat_inst = nc.gpsimd.ap_gather(
                    qgat[:], tbl_q, idxw[:, g * TCH:(g + 1) * TCH],
                    P, TQ, 4, NIC)
                add_dep_helper(gat_inst.ins, lib_inst.ins, sync=True,
                               reason="ap_gather needs library loaded")
                # out[p, 16t+c, r] holds, for every partition p of group k, the
                # quad bytes of edge (16k+c, t).  Partition p wants c == p%16:
                # dense mask-select on full partitions.
                qbf = pool.tile([P, NIC, 4], bf, tag="qbf")
                nc.vector.tensor_copy(out=qbf[:], in_=qgat[:])
                qv = qbf[:].rearrange("p (t c) r -> p t c r", c=16)
                nc.vector.tensor_tensor(
                    out=qv,
                    in0=qv,
                    in1=smask_t[:][:, None, :, None].to_broadcast([P, TCH, 16, 4]),
                    op=AO.mult)
                # reduce over c (strided innermost view): [p, t, r, c]
                wsel = pool.tile([P, TCH, 4], bf, tag="wsel")
                qcv = qbf[:].rearrange("p (t c) r -> p t r c", c=16)
                nc.vector.tensor_reduce(out=wsel[:], in_=qcv,
                                        op=AO.add, axis=mybir.AxisListType.X)
                # select quad byte r = rem
                maskr = pool.tile([P, TCH, 4], bf, tag="maskr")
                nc.vector.tensor_tensor(
                    out=maskr[:],
                    in0=rem[:, gsl][:, :, None].to_broadcast([P, TCH, 4]),
                    in1=iota4[:][:, None, :].to_broadcast([P, TCH, 4]),
                    op=AO.is_equal)
                nc.vector.tensor_tensor(out=maskr[:], in0=maskr[:],
                                        in1=wsel[:], op=AO.mult)
                nc.vector.tensor_reduce(out=dv_t[:, gsl], in_=maskr[:],
                                        op=AO.add, axis=mybir.AxisListType.X)

            # ---- expansion: out = du*A' + dv*B' + b ----
            for x in range(T // XCH):
                sl = slice(x * XCH, (x + 1) * XCH)
                xt = pool.tile([P, XCH, EMB], f32, tag="slotE")
                xo = pool.tile([P, XCH, EMB], f32, tag="slotX")
                duf = pool.tile([P, XCH], f32, tag="duf")
                dvf = pool.tile([P, XCH], f32, tag="dvf")
                nc.vector.tensor_copy(out=duf[:], in_=du_t[:, sl])
                nc.vector.tensor_copy(out=dvf[:], in_=dv_t[:, sl])
                nc.vector.tensor_tensor(
                    out=xt[:],
                    in0=duf[:][:, :, None].to_broadcast([P, XCH, EMB]),
                    in1=arep[:][:, None, :].to_broadcast([P, XCH, EMB]),
                    op=AO.mult)
                nc.vector.tensor_tensor(
                    out=xo[:],
                    in0=dvf[:][:, :, None].to_broadcast([P, XCH, EMB]),
                    in1=brep[:][:, None, :].to_broadcast([P, XCH, EMB]),
                    op=AO.mult)
                nc.vector.tensor_tensor(out=xo[:], in0=xo[:], in1=xt[:],
                                        op=AO.add)
                nc.vector.tensor_tensor(
                    out=xo[:], in0=xo[:],
                    in1=crep[:][:, None, :].to_broadcast([P, XCH, EMB]),
                    op=AO.add)
                nc.scalar.dma_start(out=out[:, sl, :], in_=xo[:])

    nc.compile()
    return nc


def _host_prep(edge_index, W, b):
    src = np.asarray(edge_index[0], dtype=np.int64).astype(np.int32)
    dst = np.asarray(edge_index[1], dtype=np.int64).astype(np.int32)
    E = src.shape[0]

    def bucketize(keys, other):
        """Distribute edges to (core, partition, col) slabs by key//BPP."""
        order = np.argsort(keys, kind="stable")
        k_s = keys[order]
        o_s = other[order] if other is not None else None
        part = (k_s // BPP).astype(np.int64)          # 0..1023 global partition
        counts = np.bincount(part, minlength=NCORES * P)
        if counts.max() > T:
            raise RuntimeError(f"slab overflow: {counts.max()} > {T}")
        starts = np.zeros(NCORES * P + 1, np.int64)
        np.cumsum(counts, out=starts[1:])
        # position of each edge within its slab
        pos_in_slab = np.arange(E, dtype=np.int64) - starts[part]
        key_arr = np.full((NCORES * P, T), -1, np.int32)
        key_arr[part, pos_in_slab] = k_s
        oth_arr = None
        if o_s is not None:
            oth_arr = np.full((NCORES * P, T), N_NODES, np.int32)
            oth_arr[part, pos_in_slab] = o_s
        # sentinel for key: base + BPP (never matches local bins 0..99)
        gp = np.arange(NCORES * P, dtype=np.int32)
        pad_val = (gp * BPP + BPP)[:, None].astype(np.int32)
        key_arr = np.where(key_arr < 0, pad_val, key_arr)
        return key_arr.reshape(NCORES, P, T), \
            (oth_arr.reshape(NCORES, P, T) if oth_arr is not None else None), \
            order, counts.reshape(NCORES, P)

    psrc_a, pdst_a, order1, counts1 = bucketize(src, dst)
    sdst_a, _, _, _ = bucketize(dst, None)

    wb = np.concatenate([np.asarray(W, np.float32),
                         np.asarray(b, np.float32)[None, :]], axis=0)
    # [A'; B'; b; 0] = mmat^T @ [W0; W1; W2; b]
    mmat = np.array([[1, 0, 0, 0],
                     [0, 1, 0, 0],
                     [1, 1, 0, 0],
                     [0, 0, 1, 0]], np.float32)
    iota_row = np.tile(np.arange(BPP, dtype=np.float32), (P, 1))
    smask_a = (np.arange(16)[None, :] == (np.arange(P) % 16)[:, None]
               ).astype(np.float32)
    in_maps = []
    for c in range(NCORES):
        basec_c = ((c * P + np.arange(P)) * BPP).astype(np.float32)[:, None]
        in_maps.append({
            "psrc": psrc_a[c], "pdst": pdst_a[c], "sdst": sdst_a[c],
            "wb": wb, "mmat": mmat, "basec": basec_c, "iotab": iota_row,
            "smask": smask_a,
        })
    return in_maps, order1, counts1


def kernel(edge_index, num_nodes, W, b):
    global _CACHE
    if "nc" not in _CACHE:
        _CACHE["nc"] = _build()
    nc = _CACHE["nc"]

    in_maps, order1, counts1 = _host_prep(edge_index, W, b)
    res = run_bass_kernel_spmd(nc, in_maps, list(range(NCORES)))

    E = np.asarray(edge_index[0]).shape[0]
    out_full = np.empty((E, EMB), np.float32)
    # rows in (core, partition, col) order, real rows only, equal order1 order
    rows = []
    for c in range(NCORES):
        o = res.results[c]["out"]          # [P, T, EMB]
        for p in range(P):
            n = counts1[c, p]
            if n:
                rows.append(o[p, :n, :])
    out_full[order1] = np.concatenate(rows, axis=0)
    return out_full



# revision 9
# speedup vs baseline: 13.0620x; 13.0620x over previous
"""DegreeAwareEdgeEncoder Trainium2 kernel (8 NeuronCores, Bass/Tile) — v2.

Strategy (all per-edge/per-node arithmetic on device; host does layout only):
  Two slab layouts, both vertex-range partitioned (102400 = 8 cores x 128
  partitions x 100 nodes): src-layout (edges bucketed+sorted by src) and
  dst-layout (by dst).  Device computes per-edge run lengths in each layout
  with one is_equal + two tensor_tensor_scan prefix scans — giving exact
  out-degree du (src-layout) and in-degree dv (dst-layout), since all edges
  of a node land in one slab row.
  The dv values then travel from dst-layout slots to src-layout slots via a
  host-precomputed (pure layout) 3-round route executed on device:
    LS0: per-partition local_scatter groups each dst-row's values by target
         core -> AllToAll exchange across the 8 cores,
    LS1: local_scatter groups each row by target partition -> DMA reshape
         through DRAM moves bucket p_s to partition p_s,
    LS2: local_scatter places values at their final src-layout column.
  Expansion out[:, j, t] = du*A[j] + dv*B[j] + b[j] (A=W0+W2, B=W1+W2 from a
  tiny on-device add) runs as 3 scalar_tensor_tensor ops per embedding column
  in bf16 (DVE fast mode), written transposed [P, EMB, T]; host inverts the
  layout permutation and the transpose on the returned rows.
"""

import numpy as np

import concourse.bass as bass
import concourse.mybir as mybir
import concourse.tile as tile
from concourse.tile_rust import add_dep_helper
from concourse import bacc
import concourse.library_config as lc
from concourse.bass_utils import run_bass_kernel_spmd

# ---- constants ----
N_NODES = 100_000
N_EDGES = 3_200_000
EMB = 32
NCORES = 8
P = 128
BPP = 100                  # nodes per partition slab
T = 3584                   # slab capacity (cols per partition)
CAP0 = 512                 # round-0 bucket capacity (per (dst-row, src-core))
CAP1 = 56                  # round-1 bucket capacity (per (row, src-partition))
W0 = NCORES * CAP0         # 4096: X0/X1 width
W1 = P * CAP1              # 7168: X2/X3 width
LS_NE = 1536               # local_scatter num_elems per call (<=2046)
PADV = 127                 # in-row pad key (never equals local ids 0..99)

NLS0 = (W0 + LS_NE - 1) // LS_NE   # 3
NLS1 = (W1 + LS_NE - 1) // LS_NE   # 5
NLS2 = (T + LS_NE - 1) // LS_NE    # 3

f32 = mybir.dt.float32
bf16 = mybir.dt.bfloat16
i16 = mybir.dt.int16
AO = mybir.AluOpType

_CACHE = {}


def _build():
    nc = bacc.Bacc("TRN2", target_bir_lowering=False, debug=False,
                   num_devices=NCORES)

    ksrc_d = nc.dram_tensor("ksrc", [P, T + 2], i16, kind="ExternalInput")
    kdst_d = nc.dram_tensor("kdst", [P, T + 2], i16, kind="ExternalInput")
    idx0_d = nc.dram_tensor("idx0", [NLS0, P, T], i16, kind="ExternalInput")
    idx1_d = nc.dram_tensor("idx1", [NLS1, P, W0], i16, kind="ExternalInput")
    idx2_d = nc.dram_tensor("idx2", [NLS2, P, W1], i16, kind="ExternalInput")
    wb_d = nc.dram_tensor("wb", [4, EMB], f32, kind="ExternalInput")
    out_d = nc.dram_tensor("out", [P, EMB, T], bf16, kind="ExternalOutput")

    a2a_in_d = nc.dram_tensor("a2a_in", [NCORES * P * CAP0], bf16)
    a2a_out_d = nc.dram_tensor("a2a_out", [NCORES * P * CAP0], bf16)
    resh_d = nc.dram_tensor("resh", [P * W1], bf16)
    abb_d = nc.dram_tensor("abb", [3 * EMB], f32)

    with tile.TileContext(nc) as tc:
        with (
            tc.tile_pool(name="main", bufs=1) as pool,
            tc.tile_pool(name="idxp", bufs=2) as idxp,
            tc.tile_pool(name="outp", bufs=2) as outp,
        ):
            lib = nc.gpsimd.load_library(lc.local_scatter)

            # ---- A/B/b coefficient rows (device) ----
            wb_t = pool.tile([1, 4 * EMB], f32)
            nc.sync.dma_start(out=wb_t[:],
                              in_=wb_d[:].rearrange("a e -> (a e)")[None, :])
            ab_t = pool.tile([1, 3 * EMB], f32)
            nc.vector.tensor_tensor(out=ab_t[:, 0:EMB], in0=wb_t[:, 0:EMB],
                                    in1=wb_t[:, 2 * EMB:3 * EMB], op=AO.add)
            nc.vector.tensor_tensor(out=ab_t[:, EMB:2 * EMB],
                                    in0=wb_t[:, EMB:2 * EMB],
                                    in1=wb_t[:, 2 * EMB:3 * EMB], op=AO.add)
            nc.vector.tensor_copy(out=ab_t[:, 2 * EMB:3 * EMB],
                                  in_=wb_t[:, 3 * EMB:4 * EMB])
            nc.sync.dma_start(out=abb_d[:][None, :], in_=ab_t[:])
            scl = pool.tile([P, 3 * EMB], f32)
            nc.sync.dma_start(out=scl[:],
                              in_=abb_d[:][None, :].to_broadcast([P, 3 * EMB]))

            def runlen(keys, cntout, tag):
                """cnt[t] = run length at slot t for a [P, T+2] padded row."""
                same = pool.tile([P, T + 1], bf16, tag="same")
                nc.vector.tensor_tensor(out=same[:], in0=keys[:, 1:T + 2],
                                        in1=keys[:, 0:T + 1], op=AO.is_equal)
                fwd = pool.tile([P, T], bf16, tag="fwd")
                nc.vector.tensor_tensor_scan(
                    out=fwd[:], data0=same[:, 0:T], data1=same[:, 0:T],
                    initial=0.0, op0=AO.mult, op1=AO.add)
                bwd = pool.tile([P, T], bf16, tag="bwd")
                nc.vector.tensor_tensor_scan(
                    out=bwd[:, ::-1], data0=same[:, T:0:-1],
                    data1=same[:, T:0:-1],
                    initial=0.0, op0=AO.mult, op1=AO.add)
                nc.vector.scalar_tensor_tensor(
                    out=cntout[:], in0=fwd[:], scalar=1.0, in1=bwd[:],
                    op0=AO.add, op1=AO.add)

            # ---- dst side: in-degree per dst-layout slot ----
            kdst = pool.tile([P, T + 2], i16)
            nc.sync.dma_start(out=kdst[:], in_=kdst_d[:])
            cnt = pool.tile([P, T], bf16)
            runlen(kdst, cnt, "d")

            def ls_round(dat, idx_dram, idx_width, width, out_tile, num_idxs,
                         tag):
                """local_scatter `dat` into out_tile using per-call idx."""
                ncalls = (width + LS_NE - 1) // LS_NE
                for k in range(ncalls):
                    lo = k * LS_NE
                    ne = min(LS_NE, width - lo)
                    idx_t = idxp.tile([P, W1], i16, tag="idx")
                    nc.sync.dma_start(out=idx_t[:, :idx_width],
                                      in_=idx_dram[k])
                    inst = nc.gpsimd.local_scatter(
                        out_tile[:, lo:lo + ne], dat[:],
                        idx_t[:, :num_idxs], P, ne, num_idxs)
                    add_dep_helper(inst.ins, lib.ins, sync=True, reason="lib")

            # ---- round 0: group by target core ----
            X0 = pool.tile([P, W0], bf16, tag="X0")
            ls_round(cnt, idx0_d, T, W0, X0, T, '0')

            # ---- AllToAll exchange ----
            nc.sync.dma_start(
                out=a2a_in_d[:].rearrange("(c p k) -> p c k", p=P, c=NCORES),
                in_=X0[:].rearrange("p (c k) -> p c k", c=NCORES))
            nc.gpsimd.collective_compute(
                "AllToAll", AO.bypass,
                replica_groups=[list(range(NCORES))],
                ins=[a2a_in_d[:]], outs=[a2a_out_d[:]])
            X1 = pool.tile([P, W0], bf16, tag="X0")
            nc.sync.dma_start(
                out=X1[:].rearrange("p (c k) -> p c k", c=NCORES),
                in_=a2a_out_d[:].rearrange("(c p k) -> p c k", p=P, c=NCORES))

            # ---- round 1: group by target partition ----
            X2 = pool.tile([P, W1], bf16, tag="X2")
            ls_round(X1, idx1_d, W0, W1, X2, W0, '1')

            # ---- reshape through DRAM: partition p_s gets bucket p_s ----
            nc.sync.dma_start(out=resh_d[:].rearrange("(p w) -> p w", p=P),
                              in_=X2[:])
            X3 = pool.tile([P, W1], bf16, tag="X2")
            nc.sync.dma_start(
                out=X3[:].rearrange("q (p r) -> q p r", r=CAP1),
                in_=resh_d[:].rearrange("(p q r) -> q p r", q=P, r=CAP1))

            # ---- round 2: final placement -> dv in src-layout ----
            dv = pool.tile([P, T], bf16, tag="dv")
            ls_round(X3, idx2_d, W1, T, dv, W1, '2')

            # ---- src side: out-degree du ----
            ksrc = pool.tile([P, T + 2], i16)
            nc.sync.dma_start(out=ksrc[:], in_=ksrc_d[:])
            du = pool.tile([P, T], bf16)
            runlen(ksrc, du, "s")

            # ---- expansion: out[:, j, :] = du*A[j] + dv*B[j] + b[j] ----
            # Scalar engine: sA = du*A[j] + b[j]; DVE: out = dv*B[j] + sA.
            NG = 4
            ident = mybir.ActivationFunctionType.Identity
            for g in range(EMB // NG):
                og = outp.tile([P, NG, T], bf16, tag="og")
                for j8 in range(NG):
                    j = NG * g + j8
                    sa = idxp.tile([P, T], bf16, tag="sa")
                    nc.scalar.activation(
                        out=sa[:], in_=du[:], func=ident,
                        scale=scl[:, j:j + 1],
                        bias=scl[:, 2 * EMB + j:2 * EMB + j + 1])
                    nc.vector.scalar_tensor_tensor(
                        out=og[:, j8, :], in0=dv[:],
                        scalar=scl[:, EMB + j:EMB + j + 1],
                        in1=sa[:], op0=AO.mult, op1=AO.add)
                nc.sync.dma_start(out=out_d[:, NG * g:NG * (g + 1), :],
                                    in_=og[:])

    nc.compile()
    return nc


def _mask_calls(idx, width):
    """Split global targets into per-call shifted idx arrays (-1 = skip)."""
    ncalls = (width + LS_NE - 1) // LS_NE
    out = np.empty((ncalls,) + idx.shape, np.int16)
    for k in range(ncalls):
        lo = k * LS_NE
        ne = min(LS_NE, width - lo)
        sh = idx.astype(np.int32) - lo
        sh[(idx < lo) | (idx >= lo + ne)] = -1
        out[k] = sh.astype(np.int16)
    return out


def _host_prep(edge_index, W, b):
    src = np.asarray(edge_index[0]).astype(np.int64)
    dst = np.asarray(edge_index[1]).astype(np.int64)
    E = src.shape[0]

    def layout(keys):
        order = np.argsort(keys, kind="stable")
        gp = keys[order] // BPP
        counts = np.bincount(gp, minlength=NCORES * P)
        if counts.max() > T:
            raise RuntimeError(f"slab overflow: {counts.max()} > {T}")
        starts = np.zeros(NCORES * P + 1, np.int64)
        np.cumsum(counts, out=starts[1:])
        tpos = np.arange(E, dtype=np.int64) - starts[gp]
        ce = np.empty(E, np.int64); pe = np.empty(E, np.int64)
        te = np.empty(E, np.int64)
        ce[order] = gp // P
        pe[order] = gp % P
        te[order] = tpos
        rows = np.full((NCORES * P, T + 2), PADV, np.int16)
        rows[gp, tpos + 1] = (keys[order] - gp * BPP).astype(np.int16)
        rows[:, 0] = -1
        rows[:, T + 1] = -2
        return order, ce, pe, te, counts.reshape(NCORES, P), \
            rows.reshape(NCORES, P, T + 2)

    order1, cs, ps, ts, n1, ksrc = layout(src)
    order2, cd, pd, td, n2, kdst = layout(dst)

    k = np.lexsort((td, pd, cd))
    ecd, epd, etd = cd[k], pd[k], td[k]
    ecs, eps, ets = cs[k], ps[k], ts[k]

    def ranks(bucket, minor):
        o = np.lexsort((minor, bucket))
        sb = bucket[o]
        r = np.empty(E, np.int64)
        r[o] = np.arange(E) - np.searchsorted(sb, sb)
        return r

    b0 = (ecd * P + epd) * NCORES + ecs
    r0 = ranks(b0, etd)
    m0 = np.bincount(b0, minlength=NCORES * P * NCORES).max()
    if m0 > CAP0:
        raise RuntimeError(f"CAP0 overflow: {m0} > {CAP0}")
    idx0 = np.full((NCORES, P, T), -1, np.int16)
    idx0[ecd, epd, etd] = (ecs * CAP0 + r0).astype(np.int16)

    j1 = ecd * CAP0 + r0
    b1 = (ecs * P + epd) * P + eps
    r1 = ranks(b1, j1)
    m1 = np.bincount(b1, minlength=NCORES * P * P).max()
    if m1 > CAP1:
        raise RuntimeError(f"CAP1 overflow: {m1} > {CAP1}")
    idx1 = np.full((NCORES, P, W0), -1, np.int16)
    idx1[ecs, epd, j1] = (eps * CAP1 + r1).astype(np.int16)

    idx2 = np.full((NCORES, P, W1), -1, np.int16)
    idx2[ecs, eps, epd * CAP1 + r1] = ets.astype(np.int16)

    wb = np.concatenate([np.asarray(W, np.float32),
                         np.asarray(b, np.float32)[None, :]], axis=0)
    in_maps = []
    for c in range(NCORES):
        in_maps.append({
            "ksrc": ksrc[c], "kdst": kdst[c],
            "idx0": _mask_calls(idx0[c], W0),
            "idx1": _mask_calls(idx1[c], W1),
            "idx2": _mask_calls(idx2[c], T),
            "wb": wb,
        })
    return in_maps, order1, n1


def kernel(edge_index, num_nodes, W, b):
    global _CACHE
    if "nc" not in _CACHE:
        _CACHE["nc"] = _build()
    nc = _CACHE["nc"]

    in_maps, order1, n1 = _host_prep(edge_index, W, b)
    res = run_bass_kernel_spmd(nc, in_maps, list(range(NCORES)))

    E = np.asarray(edge_index[0]).shape[0]
    out_full = np.empty((E, EMB), np.float32)
    rows = []
    for c in range(NCORES):
        o = np.asarray(res.results[c]["out"], np.float32)  # [P, EMB, T]
        o = o.transpose(0, 2, 1)                           # [P, T, EMB]
        for p in range(P):
            n = n1[c, p]
            if n:
                rows.append(o[p, :n, :])
    out_full[order1] = np.concatenate(rows, axis=0)
    return out_full
